# revision 26
# baseline (speedup 1.0000x reference)
"""BertAttention Trainium2 kernel (8 NeuronCores, SPMD).

Sharding: core c handles batch b = c//2 and head-half hh = c%2 (8 of 16 heads).
Each core computes q/k/v projections for its 512 head-dims over its batch's
full sequence, per-head attention (no mask, scale 1/sqrt(1024)), and a partial
o-projection over its 512 context dims.  Host sums the two partials per batch.

Device layout (per core):
  xt   [8,128,2048]  f16  hidden[b].T, d-major chunks
  wqt  [4,8,128,128] f16  w_q rows for our heads, transposed, (pair, k) chunks
  wkt  [4,8,128,128] f16  same for w_k
  wvt  [8,128,512]   f16  w_v rows transposed, k chunks
  woth [4,128,1024]  f16  w_o cols for our heads, transposed, pair chunks
  outt [8,128,2048]  f16  out_partial.T (o-major chunks)

Attention per head-pair p (heads 2p, 2p+1 local):
  QT/KT [128, 2048] = heads' q/k transposed (head on partitions 0:64 / 64:128)
  S^T tile [128k, 2, 512q]: two row-packed matmuls (K=64 at base 0 and 64),
  batched two kt at a time to cut PE row-group transition bubbles.
  exp: one activation over [128, 1024] with fused 1/32 scale -> f16
  AV: per head, lhsT = V_aug[kt][:, head, :] = [64 v-cols | 64 ones-cols],
      rhs = P^T chunk, accumulated over 16 k-tiles -> psum [128, 512];
      rows 64:128 = softmax denominator pre-broadcast across 64 partitions
      (matmul cost depends only on N, so the ones half is free).
  norm: copy den rows into aligned partitions (shifted copies are legal,
      shifted TensorTensor is not) -> one [128,512] reciprocal -> per-head
      multiply into ctx.  No broadcast matmul needed.
  oproj: pairs {0,1} and {2,3} accumulate in PSUM; one DVE copy/add each.

Scheduling: the attention inner loop is ACT(exp)-gated (~2.3us per kt-pair
vs ~1.5us of PE work), so all projection / o-projection work is emitted as
small "filler" chunks from a queue, one per kt-pair slot, keeping the PE
dense inside attention instead of as monolithic blocks between attentions.
"""

import sys

sys.path.insert(0, "/opt/trn_rl_repo")

from collections import deque

import numpy as np

B, S, D, H = 4, 2048, 1024, 16
HEAD = 64
NCORES = 8
P = 128
NQ = 512            # q free-tile width
KT_TILES = S // P   # 16 k tiles
QT_TILES = S // NQ  # 4 q tiles
DC = 8              # contraction chunks for projections (1024/128)
PAIRS = 4           # head pairs per core
WARMUP_MMS = 40     # PE warmup matmuls (cover the input-DMA window)
WARMUP_TAIL = 100   # N=128 warmup matmuls, ~56ns each

_NC_CACHE = None


def _build_nc():
    import concourse.bass as bass  # noqa: F401
    import concourse.tile as tile
    from concourse import bacc, mybir

    f32 = mybir.dt.float32
    f32r = mybir.dt.float32r
    f16 = mybir.dt.float16
    AF = mybir.ActivationFunctionType

    nc = bacc.Bacc(None)
    xt_d = nc.declare_dram_parameter("xt", [DC, P, S], f16, isOutput=False)
    wqt_d = nc.declare_dram_parameter("wqt", [PAIRS, DC, P, P], f16, isOutput=False)
    wkt_d = nc.declare_dram_parameter("wkt", [PAIRS, DC, P, P], f16, isOutput=False)
    wvt_d = nc.declare_dram_parameter("wvt", [DC, P, 512], f16, isOutput=False)
    woth_d = nc.declare_dram_parameter("woth", [PAIRS, P, D], f16, isOutput=False)
    o01_d = nc.declare_dram_parameter("out01", [D // P, P, S], f16,
                                      isOutput=True)
    o23_d = nc.declare_dram_parameter("out23", [D // P, P, S], f16,
                                      isOutput=True)

    from contextlib import ExitStack

    with tile.TileContext(nc) as tc, ExitStack() as es:
        def pool(name, bufs, space="SBUF"):
            return es.enter_context(
                tc.tile_pool(name=name, bufs=bufs, space=space))

        xt_pool = pool("xt", 1)
        wq_pool = pool("wq", 8)
        wk_pool = pool("wk", 8)
        wv_pool = pool("wv", 8)
        qt_pool = pool("qt", 2)
        kt_pool = pool("kt", 2)
        v_pool = pool("v", 1)
        pt_pool = pool("pt", 6)
        ctx_pool = pool("ctx", 1)
        wot_pool = pool("wot", 1)
        ost_pool = pool("ost", 1)
        dn_pool = pool("dn", 1)
        bc_pool = pool("bc", 2)
        on_pool = pool("on", 1)
        pp_pool = pool("pp", 2, "PSUM")
        st_pool = pool("st", 2, "PSUM")
        av_pool = pool("av", 2, "PSUM")
        st2_pool = pool("st2", 1)

        if True:
            # ones row (f16, FWL-eligible) for the denominator broadcast
            ones_h = on_pool.tile([P, P], f16, tag="onh", name="onesh")
            nc.vector.memset(ones_h[:], 1.0)

            # PE warmup while the first DMAs land: keeps HAM at 8/8 so the
            # first projection matmuls run at 2.4 GHz
            wup = on_pool.tile([P, NQ], f16, tag="wup", name="wup")
            nc.vector.memset(wup[:], 0.125)
            wups = pp_pool.tile([P, NQ], f32, tag="pp", name="wups")
            for _ in range(WARMUP_MMS):
                nc.tensor.matmul(wups[:], wup[:, 0:P], wup[:],
                                 start=True, stop=True)
            # fine-grained tail: keeps HAM at 8/8 until the input DMAs land
            # (~25us) without a big overshoot once they do
            for _ in range(WARMUP_TAIL):
                nc.tensor.matmul(wups[:, 0:P], wup[:, 0:P], wup[:, 0:P],
                                 start=True, stop=True)

            # load x^T chunks on sync/scalar queues; pair-0 weights are
            # interleaved right after the first chunk of each queue so the
            # first projection matmuls can start while x^T still streams
            xt = []
            for k in range(DC):
                t = xt_pool.tile([P, S], f16, tag=f"xt{k}", name=f"xt{k}")
                xt.append(t)
            wk0_t = []
            wq0_t = []
            for k in range(DC):
                t = wk_pool.tile([P, P], f16, tag="w", name="w")
                nc.gpsimd.dma_start(t[:], wkt_d[0, k])
                wk0_t.append(t)
                t = wq_pool.tile([P, P], f16, tag="w", name="w")
                nc.gpsimd.dma_start(t[:], wqt_d[0, k])
                wq0_t.append(t)
            # round-robin x^T across all three DMA queues, in k order so the
            # arrival-ordered projection chunks below start early
            xt_engs = (nc.sync, nc.scalar, nc.gpsimd)
            for k in range(DC):
                xt_engs[k % 3].dma_start(xt[k][:], xt_d[k])

            # V_aug: separate tiles per head-half (heads 4h..4h+3); ones col
            # per head at offset 65h+64.  One N=512 projection pass fills both.
            v_half = {0: [None] * KT_TILES, 1: [None] * KT_TILES}
            wv_t = []

            def load_wv():
                # gpsimd queue after wk0/wq0: wv is needed ~10us after them
                for k in range(DC):
                    t = wv_pool.tile([P, NQ], f16, tag="wv", name="wv")
                    nc.gpsimd.dma_start(t[:], wvt_d[k])
                    wv_t.append(t)

            def proj_v(mts):
                for mt in mts:
                    ps = pp_pool.tile([P, NQ], f32, tag="pp", name="pp")
                    for k in range(DC):
                        nc.tensor.matmul(
                            ps[:], xt[k][:, mt * P:(mt + 1) * P], wv_t[k][:],
                            start=(k == 0), stop=(k == DC - 1),
                        )
                    for half in range(2):
                        t = v_pool.tile([P, 4, P], f16, tag=f"v{half}_{mt}",
                                        name=f"v{half}_{mt}")
                        nc.vector.memset(t[:], 1.0)
                        v_half[half][mt] = t
                        src = ps[:, half * 256:half * 256 + 256].rearrange(
                            "p (h d) -> p h d", h=4)
                        nc.vector.tensor_copy(t[:, :, 0:64], src)

            def load_w(w_pool, w_dram, p, eng):
                w_t = []
                for k in range(DC):
                    t = w_pool.tile([P, P], f16, tag="w", name="w")
                    eng.dma_start(t[:], w_dram[p, k])
                    w_t.append(t)
                return w_t

            def proj_nt(out, w_t, nt):
                ps = pp_pool.tile([P, NQ], f32, tag="pp", name="pp")
                for k in range(DC):
                    nc.tensor.matmul(
                        ps[:], w_t[k][:], xt[k][:, nt * NQ:(nt + 1) * NQ],
                        start=(k == 0), stop=(k == DC - 1),
                    )
                nc.vector.tensor_copy(out[:, nt * NQ:(nt + 1) * NQ], ps[:])

            # ---- filler queue: small tagged chunks of proj/oproj work,
            # popped per attention inner-loop slot to fill ACT-gated PE
            # bubbles.  flush_through(tag) force-drains prerequisites before
            # a phase that reads them (in-order PE queues would deadlock if
            # a producer chunk were emitted after its consumer).
            fillers = deque()

            def pop_filler():
                _, fn = fillers.popleft()
                fn()

            def flush_through(tag):
                while any(t == tag for t, _ in fillers):
                    pop_filler()

            def proj_nt_chunks(tag, out, w_t, nt):
                """proj_nt split into two 4-matmul chunks."""
                state = {}

                def mk(k0):
                    def c(out=out, w_t=w_t, nt=nt, k0=k0):
                        if k0 == 0:
                            state["ps"] = pp_pool.tile([P, NQ], f32,
                                                       tag="pp", name="pp")
                        ps = state["ps"]
                        for k in range(k0, k0 + 4):
                            nc.tensor.matmul(
                                ps[:], w_t[k][:],
                                xt[k][:, nt * NQ:(nt + 1) * NQ],
                                start=(k == 0), stop=(k == DC - 1),
                            )
                        if k0 == DC - 4:
                            nc.vector.tensor_copy(
                                out[:, nt * NQ:(nt + 1) * NQ], ps[:])
                    return (tag, c)

                return [mk(k0) for k0 in range(0, DC, 4)]

            def mm_nolw(out, lhsT, rhs, tile_position):
                # matmul that reuses already-loaded PE-array weights
                # (walrus emits no LDWEIGHTS when ldweights=False)
                eng = nc.tensor
                inst = mybir.InstMatmult(
                    name=eng.bass.get_next_instruction_name(),
                    replication_resolution=0,
                    replication_shift_amnt=0,
                    replication_num_rows=0,
                    start_tensor_calc=True,
                    stop_tensor_calc=True,
                    ins=[eng.lower_ap(rhs.opt({0}), opt=False),
                         eng.lower_ap(lhsT.opt({0}), opt=False,
                                      for_matmul_weights=True)],
                    outs=[eng.lower_ap(out)],
                    tile_position=tile_position,
                    tile_size=(64, 128),
                    ldweights=False,
                )
                return eng.add_instruction(inst)

            def attention(p, QT, KT, ctx_p, post_qt=None, pre_kt=None,
                          carry_in=None, pop2_thresh=48):
                vh = v_half[p // 2]
                hl = ((2 * p) % 4, (2 * p + 1) % 4)

                def av_mms(avs, prevs):
                    # j outer: consecutive matmuls accumulate into the same
                    # psum bank, avoiding per-matmul bank alternation
                    for j in range(2):
                        for pt, kt in prevs:
                            nc.tensor.matmul(
                                avs[j][:], vh[kt][:, hl[j], :], pt[:, j, :],
                                start=(kt == 0), stop=(kt == KT_TILES - 1),
                                skip_group_check=True,
                            )

                def normalize(avs, qt):
                    # V_aug columns 64:128 are all-ones, so avs[j] rows
                    # 64:128 hold the softmax denominator already broadcast
                    # across 64 partitions -- no bc matmul needed.  Shifted
                    # COPIES are legal (the ctx copies already shift), but
                    # TensorTensor ops need aligned start partitions, so the
                    # dens are copied into place before recip + mul.
                    ctx_slices = []
                    den_b = bc_pool.tile([P, NQ], f32, tag="rc", name="denb")
                    for j in range(2):
                        ctx_slice = ctx_p[j * 64:(j + 1) * 64,
                                          qt * NQ:(qt + 1) * NQ]
                        ctx_slices.append(ctx_slice)
                        nc.vector.tensor_copy(ctx_slice, avs[j][0:64, :])
                        nc.vector.tensor_copy(den_b[j * 64:(j + 1) * 64, :],
                                              avs[j][64:P, :])
                    if fillers:
                        pop_filler()
                    rec = bc_pool.tile([P, NQ], f32, tag="rc", name="rec")
                    nc.vector.reciprocal_approx_fast(rec[:], den_b[:])
                    for j in range(2):
                        nc.vector.tensor_mul(out=ctx_slices[j],
                                             in0=ctx_slices[j],
                                             in1=rec[j * 64:(j + 1) * 64, :])

                pending = carry_in
                for qt in range(QT_TILES):
                    avs = None
                    prevs = []
                    for kt2 in range(KT_TILES // 2):
                        kts = (2 * kt2, 2 * kt2 + 1)
                        sts = []
                        for kt in kts:
                            st = st_pool.tile([P, 2, NQ], f32, tag="st",
                                              name="st")
                            nc.tensor.ldweights(
                                KT[:, kt * P:(kt + 1) * P])
                            for j in range(2):
                                h = j * 64
                                mm_nolw(
                                    st[:, j, :],
                                    KT[h:h + 64, kt * P:(kt + 1) * P],
                                    QT[h:h + 64, qt * NQ:(qt + 1) * NQ],
                                    (h, 0),
                                )
                            sts.append(st)
                        pts = []
                        for u in range(2):
                            pt = pt_pool.tile([P, 2, NQ], f16, tag="pt",
                                              name="pt")
                            nc.scalar.activation(pt[:], sts[u][:], AF.Exp,
                                                 scale=0.03125)
                            pts.append(pt)
                        # previous q-tile's tail (AVs + normalize + oproj)
                        # is emitted only after this q-tile's first QK/exp
                        # group, so the activation stream never stalls at
                        # q-tile boundaries
                        if kt2 == 0 and pending is not None:
                            pending()
                            pending = None
                        elif fillers:
                            pop_filler()
                            if len(fillers) > pop2_thresh:
                                pop_filler()
                        if pre_kt is not None and qt == 0:
                            pre_kt(list(kts))
                        if len(prevs) >= 4:
                            if avs is None:
                                avs = [av_pool.tile([P, NQ], f32, tag="av",
                                                    name=f"av{j}")
                                       for j in range(2)]
                            av_mms(avs, prevs)
                            prevs = []
                        prevs = prevs + [(pts[0], kts[0]), (pts[1], kts[1])]

                    def finish(avs=avs, prevs=prevs, qt=qt):
                        if avs is None:
                            avs = [av_pool.tile([P, NQ], f32, tag="av",
                                                name=f"av{j}")
                                   for j in range(2)]
                        av_mms(avs, prevs)
                        normalize(avs, qt)
                        if post_qt is not None:
                            post_qt(qt)
                    pending = finish
                return pending

            # pipeline: proj(0) (KT + QT nt0), then attention(p) phases with
            # proj(p+1)/oproj chunks as fillers inside the inner loops
            QTs, KTs, ctxs = {}, {}, {}
            out_sb = []
            wot_t = {}
            for _ot in range(D // P):
                _t = ost_pool.tile([P, S], f16, tag=f"ou{_ot}", name=f"ou{_ot}")
                out_sb.append(_t)

            def alloc_pair(p):
                KTs[p] = kt_pool.tile([P, S], f16, tag="t", name="kt_t")
                QTs[p] = qt_pool.tile([P, S], f16, tag="t", name="qt_t")
                ctxs[p] = ctx_pool.tile([P, S], f16, tag=f"ctx{p}",
                                        name=f"ctx{p}")

            def queue_proj_pair(p, eng):
                """Push load + proj chunks for pair p onto the filler queue."""
                state = {}
                tag = f"proj{p}"

                def load(p=p, eng=eng):
                    state["wk"] = load_w(wk_pool, wkt_d, p, eng)
                    state["wq"] = load_w(wq_pool, wqt_d, p, eng)
                    alloc_pair(p)

                fillers.append((tag, load))
                for key in ("wk", "wq"):
                    for nt in range(QT_TILES):
                        dst = KTs if key == "wk" else QTs
                        fillers.extend(
                            _proj_chunks_lazy(tag, state, key, dst, p, nt))

            def _proj_chunks_lazy(tag, state, key, dst, p, nt):
                ps_state = {}

                def mk(k0):
                    def c(k0=k0):
                        if k0 == 0:
                            ps_state["ps"] = pp_pool.tile([P, NQ], f32,
                                                          tag="pp", name="pp")
                        ps = ps_state["ps"]
                        w_t = state[key]
                        for k in range(k0, k0 + 4):
                            nc.tensor.matmul(
                                ps[:], w_t[k][:],
                                xt[k][:, nt * NQ:(nt + 1) * NQ],
                                start=(k == 0), stop=(k == DC - 1),
                            )
                        if k0 == DC - 4:
                            nc.vector.tensor_copy(
                                dst[p][:, nt * NQ:(nt + 1) * NQ], ps[:])
                    return (tag, c)

                return [mk(k0) for k0 in range(0, DC, 4)]

            def load_wot(p):
                th = wot_pool.tile([P, D], f16, tag=f"woth{p}", name=f"woth{p}")
                nc.gpsimd.dma_start(th[:], woth_d[p])
                wot_t[p] = th

            def oproj_chunk(pa, pb, qt, ots, out_dram, last=False):
                for ot in ots:
                    ps = pp_pool.tile([P, NQ], f32, tag="pp", name="pp")
                    for i, p in enumerate((pa, pb)):
                        nc.tensor.matmul(
                            ps[:], wot_t[p][:, ot * P:(ot + 1) * P],
                            ctxs[p][:, qt * NQ:(qt + 1) * NQ],
                            start=(i == 0), stop=(i == 1),
                        )
                    dst = out_sb[ot][:, qt * NQ:(qt + 1) * NQ]
                    nc.vector.tensor_copy(dst, ps[:])
                    eng = nc.gpsimd if (last and ot % 2 == 1) else nc.sync
                    eng.dma_start(out_dram[ot][:, qt * NQ:(qt + 1) * NQ], dst)

            def queue_oproj(pa, pb, out_dram, qts, last_qt=None):
                for qt in qts:
                    for ot in range(D // P):
                        fillers.append(
                            ("oproj", lambda qt=qt, ot=ot: oproj_chunk(
                                pa, pb, qt, (ot,), out_dram,
                                last=(qt == last_qt))))

            # ---- pair 0 projections emitted inline: full KT, then QT nt0/1
            # so attention(0) can start; QT nt2/3 go first on the filler
            # queue (popped during att0-qt0, well before qt2 reads them --
            # every filler chunk must be POPPED before the attention q-tile
            # that reads its output, or the in-order PE queue deadlocks)
            load_wv()
            alloc_pair(0)
            proj_nt(KTs[0], wk0_t, 0)
            proj_nt(QTs[0], wq0_t, 0)
            for nt in range(1, QT_TILES):
                fillers.extend(proj_nt_chunks("kt0", KTs[0], wk0_t, nt))
            for nt in range(1, QT_TILES):
                fillers.extend(proj_nt_chunks("qt0", QTs[0], wq0_t, nt))

            queue_proj_pair(1, nc.sync)
            pend = attention(0, QTs[0], KTs[0], ctxs[0],
                             pre_kt=lambda kts: proj_v(kts))
            flush_through("proj1")
            queue_proj_pair(2, nc.sync)
            fillers.append(("wot", lambda: (load_wot(0), load_wot(1))))
            # oproj01 qt0/qt1 pop late in att1 (ctx1 qt is normalized one
            # q-tile ahead of each pop -- checked against pop budget)
            queue_oproj(0, 1, o01_d, (0, 1))
            pend = attention(1, QTs[1], KTs[1], ctxs[1], carry_in=pend)
            flush_through("proj2")
            queue_proj_pair(3, nc.sync)
            fillers.append(("wot", lambda: (load_wot(2), load_wot(3))))
            pend = attention(2, QTs[2], KTs[2], ctxs[2], carry_in=pend)
            flush_through("proj3")
            queue_oproj(0, 1, o01_d, (2, 3))
            for _ot in range(D // P):
                fillers.append(("oproj", lambda ot=_ot: stage23(ot)))

            stage_sb = {}

            def stage23(ot):
                qt3 = QT_TILES - 1
                ps = pp_pool.tile([P, NQ], f32, tag="pp", name="pp")
                nc.tensor.matmul(
                    ps[:], wot_t[2][:, ot * P:(ot + 1) * P],
                    ctxs[2][:, qt3 * NQ:(qt3 + 1) * NQ],
                    start=True, stop=True)
                t = st2_pool.tile([P, NQ], f16, tag=f"s{ot}", name=f"s{ot}")
                stage_sb[ot] = t
                nc.vector.tensor_copy(t[:], ps[:])

            def oproj23_post(qt):
                if qt < QT_TILES - 1:
                    for ot in range(D // P):
                        fillers.append(
                            ("oproj", lambda qt=qt, ot=ot: oproj_chunk(
                                2, 3, qt, (ot,), o23_d)))
                    return
                # staged endgame for the last q-tile: the ctx2-half matmuls
                # have no dependence on the final normalize, so they fill
                # the PE while its DVE chain runs (also keeps HAM warm);
                # the tail then only needs the ctx3 matmul + a DVE add.
                qt3 = QT_TILES - 1

                def tail(ot):
                    ps = pp_pool.tile([P, NQ], f32, tag="pp", name="pp")
                    nc.tensor.matmul(
                        ps[:], wot_t[3][:, ot * P:(ot + 1) * P],
                        ctxs[3][:, qt3 * NQ:(qt3 + 1) * NQ],
                        start=True, stop=True)
                    dst = out_sb[ot][:, qt3 * NQ:(qt3 + 1) * NQ]
                    nc.vector.tensor_add(dst, ps[:], stage_sb[ot][:])
                    eng = nc.gpsimd if ot % 2 == 1 else nc.sync
                    eng.dma_start(o23_d[ot][:, qt3 * NQ:(qt3 + 1) * NQ], dst)

                for ot in range(D // P):
                    fillers.append(("oproj", lambda ot=ot: tail(ot)))

            pend = attention(3, QTs[3], KTs[3], ctxs[3], carry_in=pend,
                             post_qt=oproj23_post, pop2_thresh=6)
            pend()
            while fillers:
                pop_filler()

    nc.finalize()
    return nc


def _get_nc():
    global _NC_CACHE
    if _NC_CACHE is None:
        _NC_CACHE = _build_nc()
    return _NC_CACHE


def _make_in_maps(hidden_state, w_q, w_k, w_v, w_o):
    hidden_state = np.asarray(hidden_state, np.float32)
    w_q = np.asarray(w_q, np.float32)
    w_k = np.asarray(w_k, np.float32)
    w_v = np.asarray(w_v, np.float32)
    w_o = np.asarray(w_o, np.float32)

    in_maps = []
    for core in range(NCORES):
        b, hh = core // 2, core % 2
        rows = slice(hh * 512, (hh + 1) * 512)
        xt = hidden_state[b].T.astype(np.float16).reshape(DC, P, S)
        # w[rows].T: [1024 d, 512 c] -> (pair, k) chunks [4, 8, 128, 128]
        wqt = (w_q[rows].T.reshape(DC, P, PAIRS, P).transpose(2, 0, 1, 3)
               .astype(np.float16))
        wkt = (w_k[rows].T.reshape(DC, P, PAIRS, P).transpose(2, 0, 1, 3)
               .astype(np.float16))
        wvt = w_v[rows].T.reshape(DC, P, 512).astype(np.float16)
        woth = np.ascontiguousarray(w_o[:, rows].T.reshape(PAIRS, P, D)
                                    ).astype(np.float16)
        in_maps.append({"xt": np.ascontiguousarray(xt),
                        "wqt": np.ascontiguousarray(wqt),
                        "wkt": np.ascontiguousarray(wkt),
                        "wvt": np.ascontiguousarray(wvt),
                        "woth": woth})
    return in_maps


def _assemble(results):
    out = np.empty((B, S, D), np.float32)
    for b in range(B):
        t = np.zeros((D, S), np.float32)
        for c in (2 * b, 2 * b + 1):
            t += results[c]["out01"].reshape(D, S).astype(np.float32)
            t += results[c]["out23"].reshape(D, S).astype(np.float32)
        out[b] = t.T
    return out


def run_spmd(hidden_state, w_q, w_k, w_v, w_o, **spmd_kwargs):
    """Run the kernel; returns (output, BassKernelResults)."""
    from concourse.bass_utils import run_bass_kernel_spmd

    nc = _get_nc()
    in_maps = _make_in_maps(hidden_state, w_q, w_k, w_v, w_o)
    res = run_bass_kernel_spmd(nc, in_maps, core_ids=list(range(NCORES)),
                               **spmd_kwargs)
    return _assemble(res.results), res


def kernel(hidden_state, attention_mask=None, w_q=None, w_k=None, w_v=None,
           w_o=None):
    out, _ = run_spmd(hidden_state, w_q, w_k, w_v, w_o)
    return out


# revision 27
# speedup vs baseline: 1.0340x; 1.0340x over previous
"""BertAttention Trainium2 kernel (8 NeuronCores, SPMD).

Sharding: core c handles batch b = c//2 and head-half hh = c%2 (8 of 16 heads).
Each core computes q/k/v projections for its 512 head-dims over its batch's
full sequence, per-head attention (no mask, scale 1/sqrt(1024)), and a partial
o-projection over its 512 context dims.  Host sums the two partials per batch.

Device layout (per core):
  xt   [8,128,2048]  f16  hidden[b].T, d-major chunks
  wqt  [4,8,128,128] f16  w_q rows for our heads, transposed, (pair, k) chunks
  wkt  [4,8,128,128] f16  same for w_k
  wvt  [8,128,512]   f16  w_v rows transposed, k chunks
  woth [4,128,1024]  f16  w_o cols for our heads, transposed, pair chunks
  outt [8,128,2048]  f16  out_partial.T (o-major chunks)

Attention per head-pair p (heads 2p, 2p+1 local):
  QT/KT [128, 2048] = heads' q/k transposed (head on partitions 0:64 / 64:128)
  S^T tile [128k, 2, 512q]: two row-packed matmuls (K=64 at base 0 and 64),
  batched two kt at a time to cut PE row-group transition bubbles.
  exp: one activation over [128, 1024] with fused 1/32 scale -> f16
  AV: per head, lhsT = V_aug[kt][:, head, :] = [64 v-cols | 64 ones-cols],
      rhs = P^T chunk, accumulated over 16 k-tiles -> psum [128, 512];
      rows 64:128 = softmax denominator pre-broadcast across 64 partitions
      (matmul cost depends only on N, so the ones half is free).
  norm: copy den rows into aligned partitions (shifted copies are legal,
      shifted TensorTensor is not) -> one [128,512] reciprocal -> per-head
      multiply into ctx.  No broadcast matmul needed.
  oproj: pairs {0,1} and {2,3} accumulate in PSUM; one DVE copy/add each.

Scheduling: the attention inner loop is ACT(exp)-gated (~2.3us per kt-pair
vs ~1.5us of PE work), so all projection / o-projection work is emitted as
small "filler" chunks from a queue, one per kt-pair slot, keeping the PE
dense inside attention instead of as monolithic blocks between attentions.
"""

import sys

sys.path.insert(0, "/opt/trn_rl_repo")

from collections import deque

import numpy as np

B, S, D, H = 4, 2048, 1024, 16
HEAD = 64
NCORES = 8
P = 128
NQ = 512            # q free-tile width
KT_TILES = S // P   # 16 k tiles
QT_TILES = S // NQ  # 4 q tiles
DC = 8              # contraction chunks for projections (1024/128)
PAIRS = 4           # head pairs per core
WARMUP_MMS = 40     # PE warmup matmuls (cover the input-DMA window)
WARMUP_TAIL = 100   # N=128 warmup matmuls, ~56ns each

_NC_CACHE = None


def _build_nc():
    import concourse.bass as bass  # noqa: F401
    import concourse.tile as tile
    from concourse import bacc, mybir

    f32 = mybir.dt.float32
    f32r = mybir.dt.float32r
    f16 = mybir.dt.float16
    AF = mybir.ActivationFunctionType

    nc = bacc.Bacc(None)
    xt_d = nc.declare_dram_parameter("xt", [DC, P, S], f16, isOutput=False)
    wqt_d = nc.declare_dram_parameter("wqt", [PAIRS, DC, P, P], f16, isOutput=False)
    wkt_d = nc.declare_dram_parameter("wkt", [PAIRS, DC, P, P], f16, isOutput=False)
    wvt_d = nc.declare_dram_parameter("wvt", [DC, P, 512], f16, isOutput=False)
    woth_d = nc.declare_dram_parameter("woth", [PAIRS, P, D], f16, isOutput=False)
    o01_d = nc.declare_dram_parameter("out01", [D // P, P, S], f16,
                                      isOutput=True)
    o23_d = nc.declare_dram_parameter("out23", [D // P, P, S], f16,
                                      isOutput=True)

    from contextlib import ExitStack

    with tile.TileContext(nc) as tc, ExitStack() as es:
        def pool(name, bufs, space="SBUF"):
            return es.enter_context(
                tc.tile_pool(name=name, bufs=bufs, space=space))

        xt_pool = pool("xt", 1)
        wq_pool = pool("wq", 8)
        wk_pool = pool("wk", 8)
        wv_pool = pool("wv", 8)
        qt_pool = pool("qt", 2)
        kt_pool = pool("kt", 2)
        v_pool = pool("v", 1)
        pt_pool = pool("pt", 6)
        ctx_pool = pool("ctx", 1)
        wot_pool = pool("wot", 1)
        ost_pool = pool("ost", 1)
        dn_pool = pool("dn", 1)
        bc_pool = pool("bc", 2)
        on_pool = pool("on", 1)
        pp_pool = pool("pp", 2, "PSUM")
        st_pool = pool("st", 2, "PSUM")
        av_pool = pool("av", 2, "PSUM")
        st2_pool = pool("st2", 1)

        if True:
            # ones row (f16, FWL-eligible) for the denominator broadcast
            ones_h = on_pool.tile([P, P], f16, tag="onh", name="onesh")
            nc.vector.memset(ones_h[:], 1.0)

            # PE warmup while the first DMAs land: keeps HAM at 8/8 so the
            # first projection matmuls run at 2.4 GHz
            wup = on_pool.tile([P, NQ], f16, tag="wup", name="wup")
            nc.vector.memset(wup[:], 0.125)
            wups = pp_pool.tile([P, NQ], f32, tag="pp", name="wups")
            for _ in range(WARMUP_MMS):
                nc.tensor.matmul(wups[:], wup[:, 0:P], wup[:],
                                 start=True, stop=True)
            # fine-grained tail: keeps HAM at 8/8 until the input DMAs land
            # (~25us) without a big overshoot once they do
            for _ in range(WARMUP_TAIL):
                nc.tensor.matmul(wups[:, 0:P], wup[:, 0:P], wup[:, 0:P],
                                 start=True, stop=True)

            # load x^T chunks on sync/scalar queues; pair-0 weights are
            # interleaved right after the first chunk of each queue so the
            # first projection matmuls can start while x^T still streams
            xt = []
            for k in range(DC):
                t = xt_pool.tile([P, S], f16, tag=f"xt{k}", name=f"xt{k}")
                xt.append(t)
            wk0_t = []
            wq0_t = []
            for k in range(DC):
                t = wk_pool.tile([P, P], f16, tag="w", name="w")
                nc.gpsimd.dma_start(t[:], wkt_d[0, k])
                wk0_t.append(t)
                t = wq_pool.tile([P, P], f16, tag="w", name="w")
                nc.gpsimd.dma_start(t[:], wqt_d[0, k])
                wq0_t.append(t)
            # round-robin x^T across all three DMA queues, in k order so the
            # arrival-ordered projection chunks below start early
            xt_engs = (nc.sync, nc.scalar, nc.gpsimd)
            for k in range(DC):
                xt_engs[k % 3].dma_start(xt[k][:], xt_d[k])

            # V_aug: separate tiles per head-half (heads 4h..4h+3); ones col
            # per head at offset 65h+64.  One N=512 projection pass fills both.
            v_half = {0: [None] * KT_TILES, 1: [None] * KT_TILES}
            wv_t = []

            def load_wv():
                # gpsimd queue after wk0/wq0: wv is needed ~10us after them
                for k in range(DC):
                    t = wv_pool.tile([P, NQ], f16, tag="wv", name="wv")
                    nc.gpsimd.dma_start(t[:], wvt_d[k])
                    wv_t.append(t)

            def proj_v(mts):
                for mt in mts:
                    ps = pp_pool.tile([P, NQ], f32, tag="pp", name="pp")
                    for k in range(DC):
                        nc.tensor.matmul(
                            ps[:], xt[k][:, mt * P:(mt + 1) * P], wv_t[k][:],
                            start=(k == 0), stop=(k == DC - 1),
                        )
                    for half in range(2):
                        t = v_pool.tile([P, 4, P], f16, tag=f"v{half}_{mt}",
                                        name=f"v{half}_{mt}")
                        nc.vector.memset(t[:], 1.0)
                        v_half[half][mt] = t
                        src = ps[:, half * 256:half * 256 + 256].rearrange(
                            "p (h d) -> p h d", h=4)
                        nc.vector.tensor_copy(t[:, :, 0:64], src)

            def load_w(w_pool, w_dram, p, eng):
                w_t = []
                for k in range(DC):
                    t = w_pool.tile([P, P], f16, tag="w", name="w")
                    eng.dma_start(t[:], w_dram[p, k])
                    w_t.append(t)
                return w_t

            def proj_nt(out, w_t, nt):
                ps = pp_pool.tile([P, NQ], f32, tag="pp", name="pp")
                for k in range(DC):
                    nc.tensor.matmul(
                        ps[:], w_t[k][:], xt[k][:, nt * NQ:(nt + 1) * NQ],
                        start=(k == 0), stop=(k == DC - 1),
                    )
                nc.vector.tensor_copy(out[:, nt * NQ:(nt + 1) * NQ], ps[:])

            # ---- filler queue: small tagged chunks of proj/oproj work,
            # popped per attention inner-loop slot to fill ACT-gated PE
            # bubbles.  flush_through(tag) force-drains prerequisites before
            # a phase that reads them (in-order PE queues would deadlock if
            # a producer chunk were emitted after its consumer).
            fillers = deque()

            def pop_filler():
                _, fn = fillers.popleft()
                fn()

            def flush_through(tag):
                while any(t == tag for t, _ in fillers):
                    pop_filler()

            def proj_nt_chunks(tag, out, w_t, nt):
                """proj_nt split into two 4-matmul chunks."""
                state = {}

                def mk(k0):
                    def c(out=out, w_t=w_t, nt=nt, k0=k0):
                        if k0 == 0:
                            state["ps"] = pp_pool.tile([P, NQ], f32,
                                                       tag="pp", name="pp")
                        ps = state["ps"]
                        for k in range(k0, k0 + 4):
                            nc.tensor.matmul(
                                ps[:], w_t[k][:],
                                xt[k][:, nt * NQ:(nt + 1) * NQ],
                                start=(k == 0), stop=(k == DC - 1),
                            )
                        if k0 == DC - 4:
                            nc.vector.tensor_copy(
                                out[:, nt * NQ:(nt + 1) * NQ], ps[:])
                    return (tag, c)

                return [mk(k0) for k0 in range(0, DC, 4)]

            def attention(p, QT, KT, ctx_p, post_qt=None, pre_kt=None,
                          carry_in=None, pop2_thresh=48):
                vh = v_half[p // 2]
                hl = ((2 * p) % 4, (2 * p + 1) % 4)

                def av_mms(avs, prevs):
                    # j outer: consecutive matmuls accumulate into the same
                    # psum bank, avoiding per-matmul bank alternation
                    for j in range(2):
                        for pt, kt in prevs:
                            nc.tensor.matmul(
                                avs[j][:], vh[kt][:, hl[j], :], pt[:, j, :],
                                start=(kt == 0), stop=(kt == KT_TILES - 1),
                                skip_group_check=True,
                            )

                def normalize(avs, qt):
                    # V_aug columns 64:128 are all-ones, so avs[j] rows
                    # 64:128 hold the softmax denominator already broadcast
                    # across 64 partitions -- no bc matmul needed.  Shifted
                    # COPIES are legal (the ctx copies already shift), but
                    # TensorTensor ops need aligned start partitions, so the
                    # dens are copied into place before recip + mul.
                    ctx_slices = []
                    den_b = bc_pool.tile([P, NQ], f32, tag="rc", name="denb")
                    for j in range(2):
                        ctx_slice = ctx_p[j * 64:(j + 1) * 64,
                                          qt * NQ:(qt + 1) * NQ]
                        ctx_slices.append(ctx_slice)
                        nc.vector.tensor_copy(ctx_slice, avs[j][0:64, :])
                        nc.vector.tensor_copy(den_b[j * 64:(j + 1) * 64, :],
                                              avs[j][64:P, :])
                    if fillers:
                        pop_filler()
                    rec = bc_pool.tile([P, NQ], f32, tag="rc", name="rec")
                    nc.vector.reciprocal_approx_fast(rec[:], den_b[:])
                    for j in range(2):
                        nc.vector.tensor_mul(out=ctx_slices[j],
                                             in0=ctx_slices[j],
                                             in1=rec[j * 64:(j + 1) * 64, :])

                pending = carry_in
                for qt in range(QT_TILES):
                    avs = None
                    prevs = []
                    for kt2 in range(KT_TILES // 2):
                        kts = (2 * kt2, 2 * kt2 + 1)
                        sts = []
                        for kt in kts:
                            st = st_pool.tile([P, 2, NQ], f32, tag="st",
                                              name="st")
                            for j in range(2):
                                h = j * 64
                                nc.tensor.matmul(
                                    st[:, j, :],
                                    KT[h:h + 64, kt * P:(kt + 1) * P],
                                    QT[h:h + 64, qt * NQ:(qt + 1) * NQ],
                                    start=True, stop=True,
                                )
                            sts.append(st)
                        pts = []
                        for u in range(2):
                            pt = pt_pool.tile([P, 2, NQ], f16, tag="pt",
                                              name="pt")
                            nc.scalar.activation(pt[:], sts[u][:], AF.Exp,
                                                 scale=0.03125)
                            pts.append(pt)
                        # previous q-tile's tail (AVs + normalize + oproj)
                        # is emitted only after this q-tile's first QK/exp
                        # group, so the activation stream never stalls at
                        # q-tile boundaries
                        if kt2 == 0 and pending is not None:
                            pending()
                            pending = None
                        elif fillers:
                            pop_filler()
                            if len(fillers) > pop2_thresh:
                                pop_filler()
                        if pre_kt is not None and qt == 0:
                            pre_kt(list(kts))
                        if len(prevs) >= 4:
                            if avs is None:
                                avs = [av_pool.tile([P, NQ], f32, tag="av",
                                                    name=f"av{j}")
                                       for j in range(2)]
                            av_mms(avs, prevs)
                            prevs = []
                        prevs = prevs + [(pts[0], kts[0]), (pts[1], kts[1])]

                    def finish(avs=avs, prevs=prevs, qt=qt):
                        if avs is None:
                            avs = [av_pool.tile([P, NQ], f32, tag="av",
                                                name=f"av{j}")
                                   for j in range(2)]
                        av_mms(avs, prevs)
                        normalize(avs, qt)
                        if post_qt is not None:
                            post_qt(qt)
                    pending = finish
                return pending

            # pipeline: proj(0) (KT + QT nt0), then attention(p) phases with
            # proj(p+1)/oproj chunks as fillers inside the inner loops
            QTs, KTs, ctxs = {}, {}, {}
            out_sb = []
            wot_t = {}
            for _ot in range(D // P):
                _t = ost_pool.tile([P, S], f16, tag=f"ou{_ot}", name=f"ou{_ot}")
                out_sb.append(_t)

            def alloc_pair(p):
                KTs[p] = kt_pool.tile([P, S], f16, tag="t", name="kt_t")
                QTs[p] = qt_pool.tile([P, S], f16, tag="t", name="qt_t")
                ctxs[p] = ctx_pool.tile([P, S], f16, tag=f"ctx{p}",
                                        name=f"ctx{p}")

            def queue_proj_pair(p, eng):
                """Push load + proj chunks for pair p onto the filler queue."""
                state = {}
                tag = f"proj{p}"

                def load(p=p, eng=eng):
                    state["wk"] = load_w(wk_pool, wkt_d, p, eng)
                    state["wq"] = load_w(wq_pool, wqt_d, p, eng)
                    alloc_pair(p)

                fillers.append((tag, load))
                for key in ("wk", "wq"):
                    for nt in range(QT_TILES):
                        dst = KTs if key == "wk" else QTs
                        fillers.extend(
                            _proj_chunks_lazy(tag, state, key, dst, p, nt))

            def _proj_chunks_lazy(tag, state, key, dst, p, nt):
                ps_state = {}

                def mk(k0):
                    def c(k0=k0):
                        if k0 == 0:
                            ps_state["ps"] = pp_pool.tile([P, NQ], f32,
                                                          tag="pp", name="pp")
                        ps = ps_state["ps"]
                        w_t = state[key]
                        for k in range(k0, k0 + 4):
                            nc.tensor.matmul(
                                ps[:], w_t[k][:],
                                xt[k][:, nt * NQ:(nt + 1) * NQ],
                                start=(k == 0), stop=(k == DC - 1),
                            )
                        if k0 == DC - 4:
                            nc.vector.tensor_copy(
                                dst[p][:, nt * NQ:(nt + 1) * NQ], ps[:])
                    return (tag, c)

                return [mk(k0) for k0 in range(0, DC, 4)]

            def load_wot(p):
                th = wot_pool.tile([P, D], f16, tag=f"woth{p}", name=f"woth{p}")
                nc.gpsimd.dma_start(th[:], woth_d[p])
                wot_t[p] = th

            def oproj_chunk(pa, pb, qt, ots, out_dram, last=False):
                for ot in ots:
                    ps = pp_pool.tile([P, NQ], f32, tag="pp", name="pp")
                    for i, p in enumerate((pa, pb)):
                        nc.tensor.matmul(
                            ps[:], wot_t[p][:, ot * P:(ot + 1) * P],
                            ctxs[p][:, qt * NQ:(qt + 1) * NQ],
                            start=(i == 0), stop=(i == 1),
                        )
                    dst = out_sb[ot][:, qt * NQ:(qt + 1) * NQ]
                    nc.vector.tensor_copy(dst, ps[:])
                    eng = nc.gpsimd if (last and ot % 2 == 1) else nc.sync
                    eng.dma_start(out_dram[ot][:, qt * NQ:(qt + 1) * NQ], dst)

            def queue_oproj(pa, pb, out_dram, qts, last_qt=None):
                for qt in qts:
                    for ot in range(D // P):
                        fillers.append(
                            ("oproj", lambda qt=qt, ot=ot: oproj_chunk(
                                pa, pb, qt, (ot,), out_dram,
                                last=(qt == last_qt))))

            # ---- pair 0 projections emitted inline: full KT, then QT nt0/1
            # so attention(0) can start; QT nt2/3 go first on the filler
            # queue (popped during att0-qt0, well before qt2 reads them --
            # every filler chunk must be POPPED before the attention q-tile
            # that reads its output, or the in-order PE queue deadlocks)
            load_wv()
            alloc_pair(0)
            proj_nt(KTs[0], wk0_t, 0)
            proj_nt(QTs[0], wq0_t, 0)
            for nt in range(1, QT_TILES):
                fillers.extend(proj_nt_chunks("kt0", KTs[0], wk0_t, nt))
            for nt in range(1, QT_TILES):
                fillers.extend(proj_nt_chunks("qt0", QTs[0], wq0_t, nt))

            queue_proj_pair(1, nc.sync)
            pend = attention(0, QTs[0], KTs[0], ctxs[0],
                             pre_kt=lambda kts: proj_v(kts))
            flush_through("proj1")
            queue_proj_pair(2, nc.sync)
            fillers.append(("wot", lambda: (load_wot(0), load_wot(1))))
            # oproj01 qt0/qt1 pop late in att1 (ctx1 qt is normalized one
            # q-tile ahead of each pop -- checked against pop budget)
            queue_oproj(0, 1, o01_d, (0, 1))
            pend = attention(1, QTs[1], KTs[1], ctxs[1], carry_in=pend)
            flush_through("proj2")
            queue_proj_pair(3, nc.sync)
            fillers.append(("wot", lambda: (load_wot(2), load_wot(3))))
            pend = attention(2, QTs[2], KTs[2], ctxs[2], carry_in=pend)
            flush_through("proj3")
            queue_oproj(0, 1, o01_d, (2, 3))
            for _ot in range(D // P):
                fillers.append(("oproj", lambda ot=_ot: stage23(ot)))

            stage_sb = {}

            def stage23(ot):
                qt3 = QT_TILES - 1
                ps = pp_pool.tile([P, NQ], f32, tag="pp", name="pp")
                nc.tensor.matmul(
                    ps[:], wot_t[2][:, ot * P:(ot + 1) * P],
                    ctxs[2][:, qt3 * NQ:(qt3 + 1) * NQ],
                    start=True, stop=True)
                t = st2_pool.tile([P, NQ], f16, tag=f"s{ot}", name=f"s{ot}")
                stage_sb[ot] = t
                nc.vector.tensor_copy(t[:], ps[:])

            def oproj23_post(qt):
                if qt < QT_TILES - 1:
                    for ot in range(D // P):
                        fillers.append(
                            ("oproj", lambda qt=qt, ot=ot: oproj_chunk(
                                2, 3, qt, (ot,), o23_d)))
                    return
                # staged endgame for the last q-tile: the ctx2-half matmuls
                # have no dependence on the final normalize, so they fill
                # the PE while its DVE chain runs (also keeps HAM warm);
                # the tail then only needs the ctx3 matmul + a DVE add.
                qt3 = QT_TILES - 1

                def tail(ot):
                    ps = pp_pool.tile([P, NQ], f32, tag="pp", name="pp")
                    nc.tensor.matmul(
                        ps[:], wot_t[3][:, ot * P:(ot + 1) * P],
                        ctxs[3][:, qt3 * NQ:(qt3 + 1) * NQ],
                        start=True, stop=True)
                    dst = out_sb[ot][:, qt3 * NQ:(qt3 + 1) * NQ]
                    nc.vector.tensor_add(dst, ps[:], stage_sb[ot][:])
                    eng = nc.gpsimd if ot % 2 == 1 else nc.sync
                    eng.dma_start(o23_d[ot][:, qt3 * NQ:(qt3 + 1) * NQ], dst)

                for ot in range(D // P):
                    fillers.append(("oproj", lambda ot=ot: tail(ot)))

            pend = attention(3, QTs[3], KTs[3], ctxs[3], carry_in=pend,
                             post_qt=oproj23_post, pop2_thresh=6)
            pend()
            while fillers:
                pop_filler()

    nc.finalize()
    return nc


def _get_nc():
    global _NC_CACHE
    if _NC_CACHE is None:
        _NC_CACHE = _build_nc()
    return _NC_CACHE


def _make_in_maps(hidden_state, w_q, w_k, w_v, w_o):
    hidden_state = np.asarray(hidden_state, np.float32)
    w_q = np.asarray(w_q, np.float32)
    w_k = np.asarray(w_k, np.float32)
    w_v = np.asarray(w_v, np.float32)
    w_o = np.asarray(w_o, np.float32)

    in_maps = []
    for core in range(NCORES):
        b, hh = core // 2, core % 2
        rows = slice(hh * 512, (hh + 1) * 512)
        xt = hidden_state[b].T.astype(np.float16).reshape(DC, P, S)
        # w[rows].T: [1024 d, 512 c] -> (pair, k) chunks [4, 8, 128, 128]
        wqt = (w_q[rows].T.reshape(DC, P, PAIRS, P).transpose(2, 0, 1, 3)
               .astype(np.float16))
        wkt = (w_k[rows].T.reshape(DC, P, PAIRS, P).transpose(2, 0, 1, 3)
               .astype(np.float16))
        wvt = w_v[rows].T.reshape(DC, P, 512).astype(np.float16)
        woth = np.ascontiguousarray(w_o[:, rows].T.reshape(PAIRS, P, D)
                                    ).astype(np.float16)
        in_maps.append({"xt": np.ascontiguousarray(xt),
                        "wqt": np.ascontiguousarray(wqt),
                        "wkt": np.ascontiguousarray(wkt),
                        "wvt": np.ascontiguousarray(wvt),
                        "woth": woth})
    return in_maps


def _assemble(results):
    out = np.empty((B, S, D), np.float32)
    for b in range(B):
        t = np.zeros((D, S), np.float32)
        for c in (2 * b, 2 * b + 1):
            t += results[c]["out01"].reshape(D, S).astype(np.float32)
            t += results[c]["out23"].reshape(D, S).astype(np.float32)
        out[b] = t.T
    return out


def run_spmd(hidden_state, w_q, w_k, w_v, w_o, **spmd_kwargs):
    """Run the kernel; returns (output, BassKernelResults)."""
    from concourse.bass_utils import run_bass_kernel_spmd

    nc = _get_nc()
    in_maps = _make_in_maps(hidden_state, w_q, w_k, w_v, w_o)
    res = run_bass_kernel_spmd(nc, in_maps, core_ids=list(range(NCORES)),
                               **spmd_kwargs)
    return _assemble(res.results), res


def kernel(hidden_state, attention_mask=None, w_q=None, w_k=None, w_v=None,
           w_o=None):
    out, _ = run_spmd(hidden_state, w_q, w_k, w_v, w_o)
    return out
